# revision 21
# baseline (speedup 1.0000x reference)
"""Single-head attention on 8 NeuronCores, data-parallel over batch.

Per core (one batch item): x [T, D] with T=2048, D=1024.
    q = x@Wq.T, k = x@Wk.T, v = x@Wv.T
    score = q@k.T / sqrt(T); attn = softmax(score); out = (attn@v)@Wo.T

Weight folding (host-side, fp32): the kernel never materializes q, k or
the output projection. With M = Wq.T@Wk and N = (Wo@Wv).T:
    score = (x@M) @ x.T          (q/k projections fold into one)
    out   = attn @ (x@N)         (Wo folds into the v projection)
Per-core work drops from 17.2 to 12.9 GMAC, and the score matmul's
stationary operand is the already-resident input x.

On-chip layout is feature-major (transposed) so no device transposes:
    zT[h,t]   = M.T @ xT                   (z = x@M)
    v'[s,h]   = xT.T @ N                   (v' = x@N)
    scoreT[s,t] = xT_slice.T @ zT
    expT      = exp(scoreT/sqrt(T))        (no max subtraction: exp <= ~55)
    denom     = partition_all_reduce(sum_s expT)  (hidden under matmuls)
    oT[h,t]   = v'_slice.T @ expT;  outT = oT * (1/denom)

All matmul operands are bf16 (host pre-rounds), accumulation fp32 in
PSUM. Stationary tiles are loaded once and reused for 2-4 matmuls of
512 moving columns; redundant ldweights are stripped post-build. The
first A@v' tile drains through a Scalar-engine copy and normalizes
late, so the PE never waits on the softmax reciprocal.
"""

import numpy as np

P = 128


def _build_attention(tc, aps, D, T):
    """Emit the per-core attention kernel into TileContext `tc`.

    aps: DRAM APs xT[D,T], mT[D,D] ([x,h] for z=x@M), nT[D,D] ([x,h] for
         v'=x@N), outT[D,T] ([h,t]).
    """
    from contextlib import ExitStack

    import concourse.mybir as mybir
    from concourse import bass_isa
    from concourse.bass import ts

    nc = tc.nc
    fp32 = mybir.dt.float32
    bf16 = mybir.dt.bfloat16
    f8 = mybir.dt.float8e4
    DR = mybir.MatmulPerfMode.DoubleRow
    Exp = mybir.ActivationFunctionType.Exp

    XO = D // P          # feature (contraction) tiles: 8
    HO = D // P          # h tiles: 8
    SO = T // P          # s tiles: 16
    TC = 512             # moving-column chunk (= one PSUM bank of fp32)
    NTC = T // TC        # 4
    HALF = SO // 2
    H2 = T // 2
    SCALE = float(1.0 / np.sqrt(np.float32(T)))

    xT, mT, nT, outT = aps["xT"], aps["mT"], aps["nT"], aps["outT"]

    with ExitStack() as top:
        persist = top.enter_context(tc.tile_pool(name="persist", bufs=1))
        xsb = persist.tile([P, XO, T], bf16, name="xsb", tag="xsb")
        zT = persist.tile([P, HO, T], bf16, name="zT", tag="zT")
        vsb = persist.tile([P, SO, D], bf16, name="vsb", tag="vsb")
        exp_pool = top.enter_context(tc.tile_pool(name="expp", bufs=SO))
        # One PSUM pool for every phase: 2 rotating [P, T] fp32 buffers
        # (4 banks each = all 8 banks), so phase transitions need no new
        # allocation and pipelining depth is uniform.
        ps = top.enter_context(tc.tile_pool(name="ps", bufs=2, space="PSUM"))

        # ---------------- phase 1: v' and z projections ---------------------
        # Both consume xT; v' runs first (stationary xT s-tiles, moving N
        # weights) while M streams in behind the first loads.
        with ExitStack() as ph1:
            w_pool = ph1.enter_context(tc.tile_pool(name="w", bufs=1))
            wn = [w_pool.tile([P, D], bf16, name=f"wn{x}", tag=f"wn{x}")
                  for x in range(XO)]
            wm = [w_pool.tile([P, D], bf16, name=f"wm{x}", tag=f"wm{x}")
                  for x in range(XO)]

            # Aggregate DMA is HBM-bound (~320 GB/s across all queues), so
            # the only ordering that matters is matching the v' x-loop's
            # consumption order: per-x (N-weight, x-chunk0) pairs round-
            # robin over the three DMA-capable queues, everything else
            # behind. M lands last (z-proj needs it ~70us in).
            queues = [nc.sync, nc.gpsimd, nc.scalar]
            for x in range(XO):
                eng = queues[x % 3]
                eng.dma_start(wn[x][:], nT[ts(x, P), :])
                eng.dma_start(xsb[:, x, ts(0, TC)], xT[ts(x, P), ts(0, TC)])
            for x in range(XO):
                queues[x % 3].dma_start(xsb[:, x, TC:], xT[ts(x, P), TC:])
            for x in range(XO):
                queues[x % 3].dma_start(wm[x][:], mT[ts(x, P), :])

            # v'[s,h] = sum_x xT[x,s].T @ N[x,h]; one weight load per (s,x),
            # two 512-col matmuls per load.
            for s in range(SO):
                vps = ps.tile([P, T], fp32, name=f"vps{s}", tag="ps")
                for x in range(XO):
                    lhsT = xsb[:, x, ts(s, P)]
                    for c in range(D // TC):
                        nc.tensor.matmul(
                            vps[:, ts(c, TC)], lhsT, wn[x][:, ts(c, TC)],
                            start=(x == 0), stop=(x == XO - 1),
                        )
                nc.vector.tensor_copy(vsb[:, s, :], vps[:, :D])

            # zT[h,t] = sum_x M[x,h].T @ xT[x,t]; one weight load per (h,x),
            # four 512-col matmuls per load.
            for h in range(HO):
                zps = ps.tile([P, T], fp32, name=f"zps{h}", tag="ps")
                for x in range(XO):
                    lhsT = wm[x][:, ts(h, P)]
                    for tcc in range(NTC):
                        nc.tensor.matmul(
                            zps[:, ts(tcc, TC)], lhsT, xsb[:, x, ts(tcc, TC)],
                            start=(x == 0), stop=(x == XO - 1),
                        )
                nc.scalar.copy(zT[:, h, :], zps[:])

        # ---------------- phase 2: scores + softmax + A@v' ------------------
        with ExitStack() as ph2:
            red = ph2.enter_context(tc.tile_pool(name="red", bufs=1))
            accs = [red.tile([P, T], fp32, name=f"acc{j}", tag=f"acc{j}")
                    for j in range(2)]
            dens = [red.tile([P, T], fp32, name=f"den{j}", tag=f"den{j}")
                    for j in range(2)]

            # fp8 side tensors: the last two h-tiles of the score
            # contraction and the last two s-tiles of the A@v' contraction
            # run as fp8e4 DoubleRow matmuls (two 128-deep k-tiles per PE
            # pass, ~2x). All values are O(1) or bounded by exp<=~55, well
            # inside e4m3 range, so plain round-to-nearest casts suffice
            # and the fp8 partials accumulate unscaled into the same PSUM
            # group as the bf16 partials. Measured end-to-end rel_l2
            # 0.0175 vs the 0.02 gate.
            q8 = ph2.enter_context(tc.tile_pool(name="q8", bufs=1))
            xs8 = q8.tile([P, 2, T], f8, name="xs8", tag="xs8")
            zt8 = q8.tile([P, 2, T], f8, name="zt8", tag="zt8")
            v8 = q8.tile([P, 2, D], f8, name="v8", tag="v8")
            e8 = q8.tile([P, 2, T], f8, name="e8", tag="e8")
            for j in range(2):
                nc.vector.tensor_copy(xs8[:, j, :], xsb[:, HO - 2 + j, :])
                nc.vector.tensor_copy(v8[:, j, :], vsb[:, SO - 2 + j, :])
                nc.vector.tensor_copy(zt8[:, j, :], zT[:, HO - 2 + j, :])

            # scoreT[s,t] = sum_h xT[h,s].T @ zT[h,t]. Stationary is the
            # resident xT; h=7 is emitted last so the final zT copy hides
            # under the h=0..6 matmuls. exp on Scalar; the softmax
            # denominator accumulates on DVE in two halves, each half
            # all-reduced across partitions on GpSimd as soon as it
            # completes so the reduce hides under the remaining matmuls.
            exps = []
            for s in range(SO):
                sps = ps.tile([P, T], fp32, name=f"sps{s}", tag="ps")
                for h in range(HO - 2):
                    lhsT = xsb[:, h, ts(s, P)]
                    for tcc in range(NTC):
                        nc.tensor.matmul(
                            sps[:, ts(tcc, TC)], lhsT, zT[:, h, ts(tcc, TC)],
                            start=(h == 0), stop=False,
                        )
                for tcc in range(NTC):
                    nc.tensor.matmul(
                        sps[:, ts(tcc, TC)], xs8[:, :, ts(s, P)],
                        zt8[:, :, ts(tcc, TC)],
                        start=False, stop=True, perf_mode=DR,
                    )
                et = exp_pool.tile([P, T], bf16, name=f"exp{s}", tag="exp")
                nc.scalar.activation(et[:], sps[:], Exp, scale=SCALE)
                exps.append(et)
                j, sj = divmod(s, HALF)
                if sj == 0:
                    nc.vector.tensor_copy(accs[j][:], et[:])
                else:
                    nc.vector.tensor_add(accs[j][:], accs[j][:], et[:])
                if s >= SO - 2:
                    nc.vector.tensor_copy(e8[:, s - (SO - 2), :], et[:])
                if sj == HALF - 1:
                    nc.gpsimd.partition_all_reduce(
                        dens[j][:], accs[j][:], channels=P,
                        reduce_op=bass_isa.ReduceOp.add,
                    )

            # 1/denom: ~13us on DVE, but nothing on the PE path waits for
            # it (h=0 below drains via Scalar). acc0 is dead after its
            # all-reduce, so the reciprocal lands there.
            nc.vector.tensor_add(dens[0][:], dens[0][:], dens[1][:])
            rc = red.tile([P, T], fp32, name="rc", tag="acc0")
            nc.vector.reciprocal(rc[:], dens[0][:])

            # oT[h,t] = sum_s v'[s,h].T @ expT[s]; one weight load per
            # (h,s), four 512-col matmuls per load. h=0 drains its PSUM
            # through a Scalar copy (no reciprocal dependency) and
            # normalizes after h=1; the others normalize on DVE straight
            # out of PSUM. h=7 accumulates into two PSUM tiles (t-halves)
            # so its normalize+store overlaps the preceding matmuls.
            osb_tags = ["acc1", "den1", "den0"]
            n_osb = 0
            pend0 = None
            for h in range(HO):
                last = h == HO - 1
                if last:
                    opsA = ps.tile([P, T], fp32, name="opsA", tag="ps")
                    opsB = ps.tile([P, T], fp32, name="opsB", tag="ps")
                else:
                    opsA = ps.tile([P, T], fp32, name=f"ops{h}", tag="ps")
                    opsB = opsA
                for s in range(SO - 2):
                    lhsT = vsb[:, s, ts(h, P)]
                    for tcc in range(NTC):
                        dst = opsA if tcc < 2 else opsB
                        nc.tensor.matmul(
                            dst[:, ts(tcc, TC)], lhsT, exps[s][:, ts(tcc, TC)],
                            start=(s == 0), stop=False,
                        )
                for tcc in range(NTC):
                    dst = opsA if tcc < 2 else opsB
                    nc.tensor.matmul(
                        dst[:, ts(tcc, TC)], v8[:, :, ts(h, P)],
                        e8[:, :, ts(tcc, TC)],
                        start=False, stop=True, perf_mode=DR,
                    )
                if h == 0:
                    pend0 = red.tile([P, T], bf16, name="osb0",
                                     tag=osb_tags[n_osb % 3])
                    n_osb += 1
                    nc.scalar.copy(pend0[:], opsA[:])
                    continue
                halves = [(opsA, 0, TC), (opsA, TC, TC), (opsB, 2 * TC, TC),
                          (opsB, 3 * TC, TC)] if last else [(opsA, 0, T)]
                for src, c0, cw in halves:
                    osb = red.tile([P, T], bf16, name=f"osb{n_osb}",
                                   tag=osb_tags[n_osb % 3])
                    n_osb += 1
                    nc.vector.tensor_mul(
                        osb[:, c0:c0 + cw], src[:, c0:c0 + cw], rc[:, c0:c0 + cw]
                    )
                    eng = nc.sync if h % 2 == 0 else nc.gpsimd
                    eng.dma_start(outT[ts(h, P), c0:c0 + cw], osb[:, c0:c0 + cw])
                if h == 1:
                    nc.vector.tensor_mul(pend0[:], pend0[:], rc[:])
                    nc.sync.dma_start(outT[ts(0, P), :], pend0[:])


def _dedupe_ldweights(nc):
    """Drop InstLdweights whose weights AP matches what the PE already has
    loaded. bass emits one ldweights per matmul; our inner loops run 2-4
    matmuls per stationary tile, so ~70% of the loads are redundant
    re-streams of the same 128x128 weights."""
    removed = 0
    for f in nc.m.functions:
        for blk in f.blocks:
            insts = blk.instructions
            cur = None
            keep = []
            for i in insts:
                tn = type(i).__name__
                if tn == "InstLdweights":
                    ap = i.ins[0]
                    k = (ap.memref, ap.offset, str(ap.ap), str(ap.dtype))
                    if k == cur:
                        removed += 1
                        continue
                    cur = k
                elif tn == "InstMatmult":
                    ap = i.ins[1]
                    assert (ap.memref, ap.offset, str(ap.ap), str(ap.dtype)) == cur
                keep.append(i)
            if len(keep) != len(insts):
                insts[:] = keep
    return removed


def build_bass(D=1024, T=2048, dedupe=True):
    import concourse.mybir as mybir
    import concourse.tile as tile
    from concourse import bacc

    fp32 = mybir.dt.float32
    bf16 = mybir.dt.bfloat16
    nc = bacc.Bacc("TRN2", debug=False)
    aps = {
        "xT": nc.dram_tensor("xT", [D, T], bf16, kind="ExternalInput")[:],
        "mT": nc.dram_tensor("mT", [D, D], bf16, kind="ExternalInput")[:],
        "nT": nc.dram_tensor("nT", [D, D], bf16, kind="ExternalInput")[:],
        "outT": nc.dram_tensor("outT", [D, T], bf16, kind="ExternalOutput")[:],
    }
    with tile.TileContext(nc) as tc:
        _build_attention(tc, aps, D=D, T=T)
    if dedupe:
        _dedupe_ldweights(nc)
    nc.compile()
    return nc


def prepare_in_maps(x, W_q, W_k, W_v, W_o):
    """Host-side weight folding + per-core input maps (bf16, transposed)."""
    import ml_dtypes

    bf16 = ml_dtypes.bfloat16
    x = np.asarray(x, dtype=np.float32)
    Wq = np.asarray(W_q, np.float32)
    Wk = np.asarray(W_k, np.float32)
    Wv = np.asarray(W_v, np.float32)
    Wo = np.asarray(W_o, np.float32)
    mT = np.ascontiguousarray((Wq.T @ Wk).astype(bf16))
    nT = np.ascontiguousarray((Wo @ Wv).T.astype(bf16))
    return [
        {
            "xT": np.ascontiguousarray(x[b].T.astype(bf16)),
            "mT": mT,
            "nT": nT,
        }
        for b in range(x.shape[0])
    ]


def kernel(x, W_q, W_k, W_v, W_o):
    from concourse import bass_utils

    in_maps = prepare_in_maps(x, W_q, W_k, W_v, W_o)
    B = len(in_maps)
    nc = build_bass()
    res = bass_utils.run_bass_kernel_spmd(nc, in_maps, core_ids=list(range(B)))
    out = np.stack([res.results[b]["outT"].T for b in range(B)])
    return np.ascontiguousarray(out.astype(np.float32))


# revision 22
# speedup vs baseline: 1.0013x; 1.0013x over previous
"""Single-head attention on 8 NeuronCores, data-parallel over batch.

Per core (one batch item): x [T, D] with T=2048, D=1024.
    q = x@Wq.T, k = x@Wk.T, v = x@Wv.T
    score = q@k.T / sqrt(T); attn = softmax(score); out = (attn@v)@Wo.T

Weight folding (host-side, fp32): the kernel never materializes q, k or
the output projection. With M = Wq.T@Wk and N = (Wo@Wv).T:
    score = (x@M) @ x.T          (q/k projections fold into one)
    out   = attn @ (x@N)         (Wo folds into the v projection)
Per-core work drops from 17.2 to 12.9 GMAC, and the score matmul's
stationary operand is the already-resident input x.

On-chip layout is feature-major (transposed) so no device transposes:
    zT[h,t]   = M.T @ xT                   (z = x@M)
    v'[s,h]   = xT.T @ N                   (v' = x@N)
    scoreT[s,t] = xT_slice.T @ zT
    expT      = exp(scoreT/sqrt(T))        (no max subtraction: exp <= ~55)
    denom     = partition_all_reduce(sum_s expT)  (hidden under matmuls)
    oT[h,t]   = v'_slice.T @ expT;  outT = oT * (1/denom)

All matmul operands are bf16 (host pre-rounds), accumulation fp32 in
PSUM. Stationary tiles are loaded once and reused for 2-4 matmuls of
512 moving columns; redundant ldweights are stripped post-build. The
first A@v' tile drains through a Scalar-engine copy and normalizes
late, so the PE never waits on the softmax reciprocal.
"""

import numpy as np

P = 128


def _build_attention(tc, aps, D, T):
    """Emit the per-core attention kernel into TileContext `tc`.

    aps: DRAM APs xT[D,T], mT[D,D] ([x,h] for z=x@M), nT[D,D] ([x,h] for
         v'=x@N), outT[D,T] ([h,t]).
    """
    from contextlib import ExitStack

    import concourse.mybir as mybir
    from concourse import bass_isa
    from concourse.bass import ts

    nc = tc.nc
    fp32 = mybir.dt.float32
    bf16 = mybir.dt.bfloat16
    f8 = mybir.dt.float8e4
    DR = mybir.MatmulPerfMode.DoubleRow
    Exp = mybir.ActivationFunctionType.Exp

    XO = D // P          # feature (contraction) tiles: 8
    HO = D // P          # h tiles: 8
    SO = T // P          # s tiles: 16
    TC = 512             # moving-column chunk (= one PSUM bank of fp32)
    NTC = T // TC        # 4
    HALF = SO // 2
    H2 = T // 2
    SCALE = float(1.0 / np.sqrt(np.float32(T)))

    xT, mT, nT, outT = aps["xT"], aps["mT"], aps["nT"], aps["outT"]

    with ExitStack() as top:
        persist = top.enter_context(tc.tile_pool(name="persist", bufs=1))
        xsb = persist.tile([P, XO, T], bf16, name="xsb", tag="xsb")
        zT = persist.tile([P, HO, T], bf16, name="zT", tag="zT")
        vsb = persist.tile([P, SO, D], bf16, name="vsb", tag="vsb")
        exp_pool = top.enter_context(tc.tile_pool(name="expp", bufs=SO))
        # One PSUM pool for every phase: 2 rotating [P, T] fp32 buffers
        # (4 banks each = all 8 banks), so phase transitions need no new
        # allocation and pipelining depth is uniform.
        ps = top.enter_context(tc.tile_pool(name="ps", bufs=2, space="PSUM"))

        # ---------------- phase 1: v' and z projections ---------------------
        # Both consume xT; v' runs first (stationary xT s-tiles, moving N
        # weights) while M streams in behind the first loads.
        with ExitStack() as ph1:
            w_pool = ph1.enter_context(tc.tile_pool(name="w", bufs=1))
            wn = [w_pool.tile([P, D], bf16, name=f"wn{x}", tag=f"wn{x}")
                  for x in range(XO)]
            wm = [w_pool.tile([P, D], bf16, name=f"wm{x}", tag=f"wm{x}")
                  for x in range(XO)]

            # Aggregate DMA is HBM-bound (~320 GB/s across all queues), so
            # the only ordering that matters is matching the v' x-loop's
            # consumption order: per-x (N-weight, x-chunk0) pairs round-
            # robin over the three DMA-capable queues, everything else
            # behind. M lands last (z-proj needs it ~70us in).
            queues = [nc.sync, nc.gpsimd, nc.scalar]
            for x in range(XO):
                eng = queues[x % 3]
                eng.dma_start(wn[x][:], nT[ts(x, P), :])
                eng.dma_start(xsb[:, x, ts(0, TC)], xT[ts(x, P), ts(0, TC)])
            for tcc in range(1, NTC):
                for x in range(XO):
                    queues[x % 3].dma_start(xsb[:, x, ts(tcc, TC)],
                                            xT[ts(x, P), ts(tcc, TC)])
            for x in range(XO):
                queues[x % 3].dma_start(wm[x][:], mT[ts(x, P), :])

            # v'[s,h] = sum_x xT[x,s].T @ N[x,h]; one weight load per (s,x),
            # two 512-col matmuls per load.
            for s in range(SO):
                vps = ps.tile([P, T], fp32, name=f"vps{s}", tag="ps")
                for x in range(XO):
                    lhsT = xsb[:, x, ts(s, P)]
                    for c in range(D // TC):
                        nc.tensor.matmul(
                            vps[:, ts(c, TC)], lhsT, wn[x][:, ts(c, TC)],
                            start=(x == 0), stop=(x == XO - 1),
                        )
                nc.vector.tensor_copy(vsb[:, s, :], vps[:, :D])

            # zT[h,t] = sum_x M[x,h].T @ xT[x,t]; one weight load per (h,x),
            # four 512-col matmuls per load.
            for h in range(HO):
                zps = ps.tile([P, T], fp32, name=f"zps{h}", tag="ps")
                for x in range(XO):
                    lhsT = wm[x][:, ts(h, P)]
                    for tcc in range(NTC):
                        nc.tensor.matmul(
                            zps[:, ts(tcc, TC)], lhsT, xsb[:, x, ts(tcc, TC)],
                            start=(x == 0), stop=(x == XO - 1),
                        )
                nc.scalar.copy(zT[:, h, :], zps[:])

        # ---------------- phase 2: scores + softmax + A@v' ------------------
        with ExitStack() as ph2:
            red = ph2.enter_context(tc.tile_pool(name="red", bufs=1))
            accs = [red.tile([P, T], fp32, name=f"acc{j}", tag=f"acc{j}")
                    for j in range(2)]
            dens = [red.tile([P, T], fp32, name=f"den{j}", tag=f"den{j}")
                    for j in range(2)]

            # fp8 side tensors: the last two h-tiles of the score
            # contraction and the last two s-tiles of the A@v' contraction
            # run as fp8e4 DoubleRow matmuls (two 128-deep k-tiles per PE
            # pass, ~2x). All values are O(1) or bounded by exp<=~55, well
            # inside e4m3 range, so plain round-to-nearest casts suffice
            # and the fp8 partials accumulate unscaled into the same PSUM
            # group as the bf16 partials. Measured end-to-end rel_l2
            # 0.0175 vs the 0.02 gate.
            q8 = ph2.enter_context(tc.tile_pool(name="q8", bufs=1))
            xs8 = q8.tile([P, 2, T], f8, name="xs8", tag="xs8")
            zt8 = q8.tile([P, 2, T], f8, name="zt8", tag="zt8")
            v8 = q8.tile([P, 2, D], f8, name="v8", tag="v8")
            e8 = q8.tile([P, 2, T], f8, name="e8", tag="e8")
            for j in range(2):
                nc.vector.tensor_copy(xs8[:, j, :], xsb[:, HO - 2 + j, :])
                nc.vector.tensor_copy(v8[:, j, :], vsb[:, SO - 2 + j, :])
                nc.vector.tensor_copy(zt8[:, j, :], zT[:, HO - 2 + j, :])

            # scoreT[s,t] = sum_h xT[h,s].T @ zT[h,t]. Stationary is the
            # resident xT; h=7 is emitted last so the final zT copy hides
            # under the h=0..6 matmuls. exp on Scalar; the softmax
            # denominator accumulates on DVE in two halves, each half
            # all-reduced across partitions on GpSimd as soon as it
            # completes so the reduce hides under the remaining matmuls.
            exps = []
            for s in range(SO):
                sps = ps.tile([P, T], fp32, name=f"sps{s}", tag="ps")
                for h in range(HO - 2):
                    lhsT = xsb[:, h, ts(s, P)]
                    for tcc in range(NTC):
                        nc.tensor.matmul(
                            sps[:, ts(tcc, TC)], lhsT, zT[:, h, ts(tcc, TC)],
                            start=(h == 0), stop=False,
                        )
                for tcc in range(NTC):
                    nc.tensor.matmul(
                        sps[:, ts(tcc, TC)], xs8[:, :, ts(s, P)],
                        zt8[:, :, ts(tcc, TC)],
                        start=False, stop=True, perf_mode=DR,
                    )
                et = exp_pool.tile([P, T], bf16, name=f"exp{s}", tag="exp")
                nc.scalar.activation(et[:], sps[:], Exp, scale=SCALE)
                exps.append(et)
                j, sj = divmod(s, HALF)
                if sj == 0:
                    nc.vector.tensor_copy(accs[j][:], et[:])
                else:
                    nc.vector.tensor_add(accs[j][:], accs[j][:], et[:])
                if s >= SO - 2:
                    nc.vector.tensor_copy(e8[:, s - (SO - 2), :], et[:])
                if sj == HALF - 1:
                    nc.gpsimd.partition_all_reduce(
                        dens[j][:], accs[j][:], channels=P,
                        reduce_op=bass_isa.ReduceOp.add,
                    )

            # 1/denom: ~13us on DVE, but nothing on the PE path waits for
            # it (h=0 below drains via Scalar). acc0 is dead after its
            # all-reduce, so the reciprocal lands there.
            nc.vector.tensor_add(dens[0][:], dens[0][:], dens[1][:])
            rc = red.tile([P, T], fp32, name="rc", tag="acc0")
            nc.vector.reciprocal(rc[:], dens[0][:])

            # oT[h,t] = sum_s v'[s,h].T @ expT[s]; one weight load per
            # (h,s), four 512-col matmuls per load. h=0 drains its PSUM
            # through a Scalar copy (no reciprocal dependency) and
            # normalizes after h=1; the others normalize on DVE straight
            # out of PSUM. h=7 accumulates into two PSUM tiles (t-halves)
            # so its normalize+store overlaps the preceding matmuls.
            osb_tags = ["acc1", "den1", "den0"]
            n_osb = 0
            pend0 = None
            for h in range(HO):
                last = h == HO - 1
                if last:
                    opsA = ps.tile([P, T], fp32, name="opsA", tag="ps")
                    opsB = ps.tile([P, T], fp32, name="opsB", tag="ps")
                else:
                    opsA = ps.tile([P, T], fp32, name=f"ops{h}", tag="ps")
                    opsB = opsA
                for s in range(SO - 2):
                    lhsT = vsb[:, s, ts(h, P)]
                    for tcc in range(NTC):
                        dst = opsA if tcc < 2 else opsB
                        nc.tensor.matmul(
                            dst[:, ts(tcc, TC)], lhsT, exps[s][:, ts(tcc, TC)],
                            start=(s == 0), stop=False,
                        )
                for tcc in range(NTC):
                    dst = opsA if tcc < 2 else opsB
                    nc.tensor.matmul(
                        dst[:, ts(tcc, TC)], v8[:, :, ts(h, P)],
                        e8[:, :, ts(tcc, TC)],
                        start=False, stop=True, perf_mode=DR,
                    )
                if h == 0:
                    pend0 = red.tile([P, T], bf16, name="osb0",
                                     tag=osb_tags[n_osb % 3])
                    n_osb += 1
                    nc.scalar.copy(pend0[:], opsA[:])
                    continue
                halves = [(opsA, 0, TC), (opsA, TC, TC), (opsB, 2 * TC, TC),
                          (opsB, 3 * TC, TC)] if last else [(opsA, 0, T)]
                for src, c0, cw in halves:
                    osb = red.tile([P, T], bf16, name=f"osb{n_osb}",
                                   tag=osb_tags[n_osb % 3])
                    n_osb += 1
                    nc.vector.tensor_mul(
                        osb[:, c0:c0 + cw], src[:, c0:c0 + cw], rc[:, c0:c0 + cw]
                    )
                    eng = nc.sync if h % 2 == 0 else nc.gpsimd
                    eng.dma_start(outT[ts(h, P), c0:c0 + cw], osb[:, c0:c0 + cw])
                if h == 1:
                    nc.vector.tensor_mul(pend0[:], pend0[:], rc[:])
                    nc.sync.dma_start(outT[ts(0, P), :], pend0[:])


def _dedupe_ldweights(nc):
    """Drop InstLdweights whose weights AP matches what the PE already has
    loaded. bass emits one ldweights per matmul; our inner loops run 2-4
    matmuls per stationary tile, so ~70% of the loads are redundant
    re-streams of the same 128x128 weights."""
    removed = 0
    for f in nc.m.functions:
        for blk in f.blocks:
            insts = blk.instructions
            cur = None
            keep = []
            for i in insts:
                tn = type(i).__name__
                if tn == "InstLdweights":
                    ap = i.ins[0]
                    k = (ap.memref, ap.offset, str(ap.ap), str(ap.dtype))
                    if k == cur:
                        removed += 1
                        continue
                    cur = k
                elif tn == "InstMatmult":
                    ap = i.ins[1]
                    assert (ap.memref, ap.offset, str(ap.ap), str(ap.dtype)) == cur
                keep.append(i)
            if len(keep) != len(insts):
                insts[:] = keep
    return removed


def build_bass(D=1024, T=2048, dedupe=True):
    import concourse.mybir as mybir
    import concourse.tile as tile
    from concourse import bacc

    fp32 = mybir.dt.float32
    bf16 = mybir.dt.bfloat16
    nc = bacc.Bacc("TRN2", debug=False)
    aps = {
        "xT": nc.dram_tensor("xT", [D, T], bf16, kind="ExternalInput")[:],
        "mT": nc.dram_tensor("mT", [D, D], bf16, kind="ExternalInput")[:],
        "nT": nc.dram_tensor("nT", [D, D], bf16, kind="ExternalInput")[:],
        "outT": nc.dram_tensor("outT", [D, T], bf16, kind="ExternalOutput")[:],
    }
    with tile.TileContext(nc) as tc:
        _build_attention(tc, aps, D=D, T=T)
    if dedupe:
        _dedupe_ldweights(nc)
    nc.compile()
    return nc


def prepare_in_maps(x, W_q, W_k, W_v, W_o):
    """Host-side weight folding + per-core input maps (bf16, transposed)."""
    import ml_dtypes

    bf16 = ml_dtypes.bfloat16
    x = np.asarray(x, dtype=np.float32)
    Wq = np.asarray(W_q, np.float32)
    Wk = np.asarray(W_k, np.float32)
    Wv = np.asarray(W_v, np.float32)
    Wo = np.asarray(W_o, np.float32)
    mT = np.ascontiguousarray((Wq.T @ Wk).astype(bf16))
    nT = np.ascontiguousarray((Wo @ Wv).T.astype(bf16))
    return [
        {
            "xT": np.ascontiguousarray(x[b].T.astype(bf16)),
            "mT": mT,
            "nT": nT,
        }
        for b in range(x.shape[0])
    ]


def kernel(x, W_q, W_k, W_v, W_o):
    from concourse import bass_utils

    in_maps = prepare_in_maps(x, W_q, W_k, W_v, W_o)
    B = len(in_maps)
    nc = build_bass()
    res = bass_utils.run_bass_kernel_spmd(nc, in_maps, core_ids=list(range(B)))
    out = np.stack([res.results[b]["outT"].T for b in range(B)])
    return np.ascontiguousarray(out.astype(np.float32))
